# revision 45
# baseline (speedup 1.0000x reference)
"""Grouped-Query Attention (B=2, S=2048, E=2048, H=16, KVH=4, D=128, causal)
as a Bass/Tile kernel on 8 Trainium2 NeuronCores.

Sharding: core c handles batch b=c//4 and kv-head-group g=c%4 (4 q heads +
1 kv head per core).  Out-proj is row-sharded: each core computes a partial
[E,S] (transposed, x1024-scaled bf16) output; host scales+sums the 4
partials per batch.

Key speed tricks vs the bf16 baseline:
 * Projections and out-proj run as fp8e4m3 DoubleRow matmuls (0.5 cy/row,
   2 contraction chunks per instruction).  Each operand is split hi/lo
   (A = fp8(A) + fp8(A - fp8(A))); main term pairs (W_hi,W_lo)x(x_hi,x_hi)
   [stride-0 broadcast], correction term pairs chunk pairs of W_hi x x_lo.
   Dropping the lo*lo term leaves ~2^-8 relative error at 0.75x the bf16
   PE cost.  Weights are pre-scaled by 64 so the fp8-lo parts stay normal.
 * Causal diagonal 512x512 super-blocks use shrinking-width score/exp/PV
   ops (512/384/256/128) instead of full width + masking.
 * Softmax denominator: probs tiles are tree-added on the DVE (bf16);
   windows 0-2 reduce across partitions on GPSIMD, the last window uses a
   ones-matmul on the PE instead (drops the slow Pool all-reduce from the
   tail-critical normalize chain; its PSUM tile borrows the then-idle
   scores pool so the final out-proj keeps both ps_proj banks).
 * v-transpose (vn tiles for PV) via the DMA XBAR transpose instead of PE
   identity-matmuls + DVE PSUM drains; the v natural scale folds into the
   PSUM->SBUF copy (tensor_scalar_mul).
 * Per-window loop (512 q columns) interleaves projection, attention and
   the PREVIOUS window's out-proj so the PE stays fed while the ACT engine
   computes exps; x is DMA'd window-by-window into a 2-window double
   buffer (frees 32KB/partition of SBUF); startup x loads issue on the
   Activation HWDGE queue in parallel with SP's weight loads.
 * Final out-proj: last 4 chunks get their own half-size DMAs so the tail
   transfer departs earlier.
"""
import sys

for _p in ("/opt/trn_rl_repo", "/root/.axon_site/_ro/trn_rl_repo"):
    if _p not in sys.path:
        sys.path.append(_p)

import numpy as np
import ml_dtypes

import concourse.bass as bass
import concourse.bass_isa as bass_isa
import concourse.mybir as mybir
import concourse.tile as tile
from concourse import bacc, bass_utils

B, S, E = 2, 2048, 2048
H, KVH = 16, 4
D = E // H              # 128
HPC = 4                 # q heads per core
SCALE = 1.0 / float(np.sqrt(D))
P = 128
NQ = 512                # q window
NW = S // NQ            # 4 windows
WS = 64.0               # weight pre-scale (wqkv, wo)
VS = 0.25               # v-natural extra scale -> v carries 64*0.25 = 16
OUTSCALE = 1.0 / (WS * WS * VS)   # host-side final unscale (1/1024)
EXPSCALE = SCALE / (WS * WS)
BF = mybir.dt.bfloat16
F8 = mybir.dt.float8e4
F32 = mybir.dt.float32
DR = mybir.MatmulPerfMode.DoubleRow
E4 = ml_dtypes.float8_e4m3
BF_NP = ml_dtypes.bfloat16

_CACHE = {}


def _build():
    nc = bacc.Bacc("TRN2", target_bir_lowering=False, debug=False, num_devices=8)
    xhiT = nc.dram_tensor("xhiT", [E, S], F8, kind="ExternalInput").ap()
    xloT = nc.dram_tensor("xloT", [E, S], F8, kind="ExternalInput").ap()
    wmain = nc.dram_tensor("wmain", [E, 2, 768], F8, kind="ExternalInput").ap()
    womain = nc.dram_tensor("womain", [512, 2, E], F8, kind="ExternalInput").ap()
    identq = nc.dram_tensor("identq", [P, P], BF, kind="ExternalInput").ap()
    tri = nc.dram_tensor("tri", [P, P], BF, kind="ExternalInput").ap()
    outT = nc.dram_tensor("outT", [E, S], BF, kind="ExternalOutput").ap()

    EK = E // P          # 16 contraction chunks
    with tile.TileContext(nc) as tc:
        with tc.tile_pool(name="persist", bufs=1) as pp, \
             tc.tile_pool(name="probs", bufs=12) as prb, \
             tc.tile_pool(name="sacc", bufs=4) as sap, \
             tc.tile_pool(name="attn", bufs=2) as atp, \
             tc.tile_pool(name="bcast", bufs=2) as bcp, \
             tc.tile_pool(name="small", bufs=2) as smp, \
             tc.tile_pool(name="outp", bufs=6) as outp, \
             tc.tile_pool(name="ps_proj", bufs=2, space="PSUM") as ps_proj, \
             tc.tile_pool(name="ps_s", bufs=2, space="PSUM") as ps_sp, \
             tc.tile_pool(name="ps_o", bufs=2, space="PSUM") as ps_op:

            # ---- persistent SBUF tiles ----
            tri_sb = pp.tile([P, P], BF, tag="tri", name="tri")
            id_sb = pp.tile([P, P], BF, tag="identq", name="identq")
            ones_sb = pp.tile([P, P], BF, tag="ones", name="ones")
            nc.gpsimd.memset(ones_sb, 1.0)

            # Weight/x storage as single big tiles so loads batch into few
            # DMA instructions.  wmain interleaves (hi,lo) per chunk so each
            # arriving chunk-pair DMA unlocks its full 3-term DR group.
            # x loads issue on the Activation HWDGE queue in parallel with
            # the SP queue's weight loads (~650ns serial issue per queue).
            # x is consumed window-by-window: double-buffer the two live
            # windows instead of holding all of S (frees 32KB/partition)
            wm_sb = pp.tile([P, EK, 2, 768], F8, tag="wm", name="wm")
            xhi_sb = pp.tile([P, EK, 2, NQ], F8, tag="xh", name="xh")
            xlo_sb = pp.tile([P, EK // 2, 2, 2, NQ], F8, tag="xl", name="xl")
            wom_sb = pp.tile([P, 4, 2, E], F8, tag="wom", name="wom")
            # wm on the SP queue, x window-0 on the Activation HWDGE queue:
            # two parallel ~650ns/DMA issue streams; the shared DMA bus then
            # delivers chunk pairs in consumption order
            for g in range(8):
                nc.sync.dma_start(
                    out=wm_sb[:, 2 * g:2 * g + 2].rearrange(
                        "p k two c -> p k (two c)"),
                    in_=wmain[g * 256:(g + 1) * 256].rearrange(
                        "(k p) two c -> p k (two c)", p=P))
                nc.scalar.dma_start(
                    out=xhi_sb[:, 2 * g:2 * g + 2, 0],
                    in_=xhiT[g * 256:(g + 1) * 256, 0:NQ].rearrange(
                        "(k p) c -> p k c", p=P))
                if g % 4 == 1:
                    gg = g // 4
                    nc.scalar.dma_start(
                        out=xlo_sb[:, 4 * gg:4 * gg + 4, :, 0].rearrange(
                            "p k two c -> p (k two) c"),
                        in_=xloT[gg * 1024:(gg + 1) * 1024, 0:NQ].rearrange(
                            "(g2 p) c -> p g2 c", p=P))

            nc.scalar.dma_start(out=tri_sb, in_=tri)
            nc.scalar.dma_start(out=id_sb, in_=identq)

            def load_wo():
                nc.sync.dma_start(
                    out=wom_sb.rearrange("p k two c -> p k (two c)"),
                    in_=womain.rearrange("(k p) two c -> p k (two c)", p=P))

            qkv_sb = [pp.tile([P, S], BF, tag=f"qkv{m}", name=f"qkv{m}") for m in range(6)]
            kT = qkv_sb[4]
            vT = qkv_sb[5]
            vn_sb = [pp.tile([P, D], BF, tag=f"vn{kc}", name=f"vn{kc}") for kc in range(S // P)]

            def load_x_window(w):
                sl = slice(w * NQ, (w + 1) * NQ)
                sb = w % 2
                for g in range(2):
                    nc.sync.dma_start(
                        out=xhi_sb[:, 8 * g:8 * g + 8, sb],
                        in_=xhiT[g * 1024:(g + 1) * 1024, sl].rearrange(
                            "(k p) c -> p k c", p=P))
                nc.sync.dma_start(
                    out=xlo_sb[:, :, :, sb].rearrange(
                        "p k two c -> p (k two) c"),
                    in_=xloT[:, sl].rearrange("(g2 p) c -> p g2 c", p=P))

            def proj_window(w, m):
                sl = slice(w * NQ, (w + 1) * NQ)
                sb = w % 2
                ps = ps_proj.tile([P, NQ], F32, tag="proj", name="proj")
                for ke in range(EK):
                    mv = xhi_sb[:, ke, sb].unsqueeze(1).broadcast_to(
                        [P, 2, NQ])
                    nc.tensor.matmul(ps, wm_sb[:, ke, :, m * P:(m + 1) * P],
                                     mv, start=(ke == 0), stop=False,
                                     perf_mode=DR)
                for k in range(EK // 2):
                    nc.tensor.matmul(
                        ps, wm_sb[:, 2 * k:2 * k + 2, 0, m * P:(m + 1) * P],
                        xlo_sb[:, k, :, sb], start=False,
                        stop=(k == EK // 2 - 1), perf_mode=DR)
                if m == 5:     # v gets its natural scale folded in here
                    nc.vector.tensor_scalar_mul(qkv_sb[m][:, sl], ps, VS)
                else:
                    nc.vector.tensor_copy(qkv_sb[m][:, sl], ps)

            def vnat_window(w):
                # transpose vT 128-chunks into vn via the DMA XBAR (frees
                # the PE transpose + DVE psum-drain the old path needed)
                for j in range(4):
                    kc = 4 * w + j
                    nc.sync.dma_start_transpose(
                        vn_sb[kc], vT[:, kc * P:(kc + 1) * P])

            _ot_pend = {}

            def outproj_chunk(w, me, ahi, alo, alt=False, single=False):
                sl = slice(w * NQ, (w + 1) * NQ)
                ps = ps_proj.tile([P, NQ], F32, tag="proj", name="proj")
                for j in range(4):
                    mv = ahi[j].unsqueeze(1).broadcast_to([P, 2, NQ])
                    nc.tensor.matmul(ps, wom_sb[:, j, :, me * P:(me + 1) * P],
                                     mv, start=(j == 0), stop=False,
                                     perf_mode=DR)
                for p_ in range(2):
                    nc.tensor.matmul(
                        ps, wom_sb[:, 2 * p_:2 * p_ + 2, 0, me * P:(me + 1) * P],
                        alo[p_], start=False, stop=(p_ == 1), perf_mode=DR)
                if single:
                    # tail chunks: own tile + own DMA so the last transfer
                    # is half-size and departs as soon as its copy lands
                    ot = outp.tile([P, NQ], BF, tag="outs", name="outs")
                    [nc.vector.tensor_copy, nc.scalar.copy][me % 2](ot, ps)
                    nc.sync.dma_start(out=outT[me * P:(me + 1) * P, sl],
                                      in_=ot)
                    return
                # pairs of me chunks share one SBUF tile and one output DMA
                if me % 2 == 0:
                    _ot_pend["t"] = outp.tile([P, 2, NQ], BF, tag="out",
                                              name="out")
                ot = _ot_pend["t"]
                sel = (me % 2) if alt else ((me // 2) % 2)
                [nc.vector.tensor_copy, nc.scalar.copy][sel](
                    ot[:, me % 2], ps)
                if me % 2 == 1:
                    nc.sync.dma_start(
                        out=outT[(me - 1) * P:(me + 1) * P, sl].rearrange(
                            "(k p) c -> p k c", p=P),
                        in_=ot)

            def attn_pair(w, h0, drain, early_free=False, mm_den=False):
                """Attention for heads h0,h0+1 interleaved per k-block pair.
                Scores/probs live in [P,2,NQ] pair tiles (2 PSUM banks) so
                one exp covers two k-blocks; PE filler thunks (next-window
                projection, previous-window out-proj) are drained between
                pair steps to keep the PE busy while ACT runs exps."""
                qsl0 = w * NQ
                heads = (h0, h0 + 1)
                npairs = 2 * w + 2
                ps_o = {h: ps_op.tile([P, NQ], F32, tag="o", name="o")
                        for h in heads}
                pend = {h: None for h in heads}   # pending group pair-tile
                racc = {h: None for h in heads}
                dtiles = {h: [] for h in heads}

                def flat(ap):
                    return ap.rearrange("p two c -> p (two c)")

                for pi in range(npairs):
                    kc0 = 2 * pi
                    for h in heads:
                        pp_s = ps_sp.tile([P, 2, NQ], F32, tag="s", name="s")
                        pr = prb.tile([P, 2, NQ], BF, tag="pr", name="pr")
                        fps, fpr = flat(pp_s), flat(pr)
                        if pi < 2 * w:          # two full-width blocks
                            for i in range(2):
                                nc.tensor.matmul(
                                    pp_s[:, i, :],
                                    kT[:, (kc0 + i) * P:(kc0 + i + 1) * P],
                                    qkv_sb[h][:, qsl0:qsl0 + NQ],
                                    start=True, stop=True)
                            nc.scalar.activation(
                                fpr, fps, mybir.ActivationFunctionType.Exp,
                                scale=EXPSCALE)
                            for i in range(2):
                                nc.tensor.matmul(ps_o[h], vn_sb[kc0 + i],
                                                 pr[:, i, :],
                                                 start=(kc0 + i == 0),
                                                 stop=False,
                                                 skip_group_check=True)
                            if pend[h] is None:
                                pend[h] = pr
                            else:               # fold group of 4 into racc
                                sa = sap.tile([P, 2, NQ], BF, tag="sa",
                                              name="sa")
                                nc.vector.tensor_add(sa, flat(pend[h]), fpr)
                                if racc[h] is None:
                                    racc[h] = sa
                                else:
                                    nc.vector.tensor_add(
                                        flat(racc[h]), flat(racc[h]),
                                        flat(sa))
                                pend[h] = None
                        elif pi == 2 * w:       # diag D1: widths 512, 384
                            nc.tensor.matmul(
                                pp_s[:, 0, :], kT[:, kc0 * P:(kc0 + 1) * P],
                                qkv_sb[h][:, qsl0:qsl0 + NQ],
                                start=True, stop=True)
                            nc.tensor.matmul(
                                pp_s[:, 1, 0:384],
                                kT[:, (kc0 + 1) * P:(kc0 + 2) * P],
                                qkv_sb[h][:, qsl0 + P:qsl0 + NQ],
                                start=True, stop=True)
                            nc.scalar.activation(
                                fpr[:, 0:896], fps[:, 0:896],
                                mybir.ActivationFunctionType.Exp,
                                scale=EXPSCALE)
                            for i in range(2):
                                nc.vector.tensor_mul(pr[:, i, 0:P],
                                                     pr[:, i, 0:P], tri_sb)
                            nc.tensor.matmul(ps_o[h], vn_sb[kc0], pr[:, 0, :],
                                             start=(kc0 == 0), stop=False,
                                             skip_group_check=True)
                            nc.tensor.matmul(ps_o[h][:, P:], vn_sb[kc0 + 1],
                                             pr[:, 1, 0:384],
                                             start=False, stop=False,
                                             skip_group_check=True)
                            dtiles[h].append(pr)
                        else:                   # diag D2: widths 256, 128
                            nc.tensor.matmul(
                                pp_s[:, 0, 0:256],
                                kT[:, kc0 * P:(kc0 + 1) * P],
                                qkv_sb[h][:, qsl0 + 2 * P:qsl0 + NQ],
                                start=True, stop=True)
                            nc.tensor.matmul(
                                pp_s[:, 1, 0:P],
                                kT[:, (kc0 + 1) * P:(kc0 + 2) * P],
                                qkv_sb[h][:, qsl0 + 3 * P:qsl0 + NQ],
                                start=True, stop=True)
                            nc.scalar.activation(
                                pr[:, 0, 0:256], pp_s[:, 0, 0:256],
                                mybir.ActivationFunctionType.Exp,
                                scale=EXPSCALE)
                            nc.scalar.activation(
                                pr[:, 1, 0:P], pp_s[:, 1, 0:P],
                                mybir.ActivationFunctionType.Exp,
                                scale=EXPSCALE)
                            for i in range(2):
                                nc.vector.tensor_mul(pr[:, i, 0:P],
                                                     pr[:, i, 0:P], tri_sb)
                            nc.tensor.matmul(ps_o[h][:, 2 * P:],
                                             vn_sb[kc0], pr[:, 0, 0:256],
                                             start=False, stop=False,
                                             skip_group_check=True)
                            nc.tensor.matmul(ps_o[h][:, 3 * P:],
                                             vn_sb[kc0 + 1], pr[:, 1, 0:P],
                                             start=False, stop=True,
                                             skip_group_check=True)
                            dtiles[h].append(pr)
                    drain()

                out = []
                # last-window pair 0: drain PV to SBUF immediately so ps_o
                # recycles for pair 1 without waiting the normalize chain
                tn = {}
                if early_free:
                    for h in heads:
                        tn[h] = atp.tile([P, NQ], F32, tag=f"tn{h % 2}",
                                         name=f"tn{h % 2}")
                        nc.vector.tensor_copy(tn[h], ps_o[h])
                for h in heads:
                    d1, d2 = dtiles[h]
                    da = d1[:, 0, :]            # diag accum, in place
                    nc.vector.tensor_add(da[:, P:], da[:, P:],
                                         flat(d1)[:, NQ:NQ + 384])
                    nc.vector.tensor_add(da[:, 2 * P:], da[:, 2 * P:],
                                         d2[:, 0, 0:256])
                    nc.vector.tensor_add(da[:, 3 * P:], da[:, 3 * P:],
                                         d2[:, 1, 0:P])
                    if racc[h] is None:
                        acc = da
                    else:
                        fa = sap.tile([P, NQ], BF, tag="fa", name="fa")
                        nc.vector.tensor_add(fa, racc[h][:, 0, :],
                                             racc[h][:, 1, :])
                        nc.vector.tensor_add(fa, fa, da)
                        acc = fa
                    if mm_den:
                        # denominator as a ones-matmul: every PSUM row gets
                        # sum-over-partitions(acc).  Cheap on PE (512 cy) and
                        # drops the slow Pool all-reduce from the last
                        # window's tail-critical normalize chain.  Borrow a
                        # scores-pool tile (idle once attention is done) so
                        # ps_proj stays free for the final out-proj chunks.
                        den_t = ps_sp.tile([P, 2, NQ], F32, tag="s",
                                           name="s")
                        den = den_t[:, 0, :]
                        nc.tensor.matmul(den, ones_sb, acc, start=True,
                                         stop=True)
                    else:
                        den = bcp.tile([P, NQ], F32, tag="ar", name="ar")
                        nc.gpsimd.partition_all_reduce(den, acc, P,
                                                       bass_isa.ReduceOp.add)
                    bc = bcp.tile([P, NQ], F32, tag="bc", name="bc")
                    nc.vector.reciprocal(bc, den)
                    t = atp.tile([P, NQ], BF, tag=f"t{h % 2}",
                                 name=f"t{h % 2}")
                    nc.vector.tensor_mul(t, tn[h] if early_free else ps_o[h],
                                         bc)
                    ahi = atp.tile([P, NQ], F8, tag=f"ahi{h}", name=f"ahi{h}")
                    nc.scalar.copy(ahi, t)
                    out.append((t, ahi))
                    drain()
                return out

            # ---------------- main window loop ----------------
            from collections import deque

            def make_drain(fillers, nsteps, boost=1.0):
                st = {"a": 0.0, "r": len(fillers) / nsteps * boost}

                def drain():
                    st["a"] += st["r"]
                    while fillers and st["a"] >= 1.0:
                        st["a"] -= 1.0
                        fillers.popleft()()
                return drain

            for m in (0, 1, 4, 5, 2, 3):
                proj_window(0, m)
            vnat_window(0)
            prev = None       # (ahi list, alo list) of previous window
            for w in range(NW):
                if w + 1 < NW:
                    load_x_window(w + 1)
                if w == 0:
                    load_wo()
                fillers = deque()
                if prev is not None:
                    pv = prev
                    for me in range(16):
                        fillers.append(
                            lambda me=me, pv=pv: outproj_chunk(
                                w - 1, me, pv[0], pv[1]))
                if w + 1 < NW:
                    for m in (0, 1, 4, 5, 2, 3):
                        fillers.append(
                            lambda m=m, w1=w + 1: proj_window(w1, m))
                    fillers.append(lambda w1=w + 1: vnat_window(w1))
                ahi_l, alo_l = [], [None, None]
                nsteps = 2 * (2 * w + 2) + 4
                last = w == NW - 1
                drain = make_drain(fillers, nsteps, boost=1.0)
                for h0 in (0, 2):
                    pair = attn_pair(w, h0, drain, mm_den=last)
                    for i, (t, ahi) in enumerate(pair):
                        h = h0 + i
                        if h % 2 == 0:
                            alo_l[h // 2] = atp.tile(
                                [P, 2, NQ], F8, tag=f"alo{h // 2}",
                                name=f"alo{h // 2}")
                        nc.vector.scalar_tensor_tensor(
                            alo_l[h // 2][:, h % 2], t, 1.0, ahi,
                            mybir.AluOpType.mult, mybir.AluOpType.subtract)
                        ahi_l.append(ahi)
                while fillers:
                    fillers.popleft()()
                prev = (ahi_l, alo_l)
            for me in range(16):
                outproj_chunk(NW - 1, me, prev[0], prev[1], single=me >= 12)

    nc.finalize()
    # The standard compile pipeline leaves the (unused) register preamble
    # in place here, which the walrus birverifier then rejects with "Reg
    # has not been allocated yet"; a second DCE pass removes it.
    nc.dce_regs()
    return nc


def _get_nc():
    if "nc" not in _CACHE:
        _CACHE["nc"] = _build()
    return _CACHE["nc"]


def _hi_lo(a):
    hi = a.astype(E4)
    lo = (a - hi.astype(np.float32)).astype(E4)
    return hi, lo


def kernel(x, Wq, Wk, Wv, Wo, _trace=False, _tmpdir=None):
    x = np.asarray(x, np.float32)
    Wq, Wk, Wv, Wo = (np.asarray(a, np.float32) for a in (Wq, Wk, Wv, Wo))
    nc = _get_nc()
    identq = np.eye(P, dtype=np.float32).astype(BF_NP)
    tri = np.triu(np.ones((P, P), np.float32)).astype(BF_NP)
    from concurrent.futures import ThreadPoolExecutor

    def _xprep(b):
        xT = np.ascontiguousarray(x[b].T)
        return _hi_lo(xT)

    with ThreadPoolExecutor(8) as tp:
        xp = list(tp.map(_xprep, range(B)))

        def _core(c):
            b, g = c // 4, c % 4
            wqkv = np.concatenate(
                [Wq[512 * g:512 * (g + 1)],
                 Wk[128 * g:128 * (g + 1)],
                 Wv[128 * g:128 * (g + 1)]], axis=0) * WS
            wqkvT = np.ascontiguousarray(wqkv.T)
            whi, wlo = _hi_lo(wqkvT)
            wmain = np.ascontiguousarray(np.stack([whi, wlo], axis=1))
            woT = np.ascontiguousarray(Wo[:, 512 * g:512 * (g + 1)].T) * WS
            wohi, wolo = _hi_lo(woT)
            womain = np.ascontiguousarray(np.stack([wohi, wolo], axis=1))
            return {
                "xhiT": xp[b][0], "xloT": xp[b][1],
                "wmain": wmain, "womain": womain,
                "identq": identq, "tri": tri,
            }
        in_maps = list(tp.map(_core, range(8)))
    res = bass_utils.run_bass_kernel_spmd(
        nc, in_maps, core_ids=list(range(8)), trace=_trace, tmpdir=_tmpdir)
    out = np.zeros((B, S, E), np.float32)
    for c in range(8):
        out[c // 4] += res.results[c]["outT"].astype(np.float32).T
    out *= OUTSCALE
    if _trace:
        return out, res
    return out



# revision 51
# speedup vs baseline: 1.0024x; 1.0024x over previous
"""Grouped-Query Attention (B=2, S=2048, E=2048, H=16, KVH=4, D=128, causal)
as a Bass/Tile kernel on 8 Trainium2 NeuronCores.

Sharding: core c handles batch b=c//4 and kv-head-group g=c%4 (4 q heads +
1 kv head per core).  Out-proj is row-sharded: each core computes a partial
[E,S] (transposed, x1024-scaled bf16) output; host scales+sums the 4
partials per batch.

Key speed tricks vs the bf16 baseline:
 * Projections and out-proj run as fp8e4m3 DoubleRow matmuls (0.5 cy/row,
   2 contraction chunks per instruction).  Each operand is split hi/lo
   (A = fp8(A) + fp8(A - fp8(A))); main term pairs (W_hi,W_lo)x(x_hi,x_hi)
   [stride-0 broadcast], correction term pairs chunk pairs of W_hi x x_lo.
   Dropping the lo*lo term leaves ~2^-8 relative error at 0.75x the bf16
   PE cost.  Weights are pre-scaled by 64 so the fp8-lo parts stay normal.
 * Causal diagonal 512x512 super-blocks use shrinking-width score/exp/PV
   ops (512/384/256/128) instead of full width + masking.
 * Softmax denominator: probs tiles are tree-added on the DVE (bf16);
   windows 0-2 reduce across partitions on GPSIMD, the last window uses a
   ones-matmul on the PE instead (drops the slow Pool all-reduce from the
   tail-critical normalize chain; its PSUM tile borrows the then-idle
   scores pool so the final out-proj keeps both ps_proj banks).
 * v-transpose (vn tiles for PV) via the DMA XBAR transpose instead of PE
   identity-matmuls + DVE PSUM drains; the v natural scale folds into the
   PSUM->SBUF copy (tensor_scalar_mul).
 * Per-window loop (512 q columns) interleaves projection, attention and
   the PREVIOUS window's out-proj so the PE stays fed while the ACT engine
   computes exps; x is DMA'd window-by-window into a 2-window double
   buffer (frees 32KB/partition of SBUF); startup x loads issue on the
   Activation HWDGE queue in parallel with SP's weight loads.
 * Final out-proj: last 4 chunks get their own half-size DMAs so the tail
   transfer departs earlier.
"""
import sys

for _p in ("/opt/trn_rl_repo", "/root/.axon_site/_ro/trn_rl_repo"):
    if _p not in sys.path:
        sys.path.append(_p)

import numpy as np
import ml_dtypes

import concourse.bass as bass
import concourse.bass_isa as bass_isa
import concourse.mybir as mybir
import concourse.tile as tile
from concourse import bacc, bass_utils

B, S, E = 2, 2048, 2048
H, KVH = 16, 4
D = E // H              # 128
HPC = 4                 # q heads per core
SCALE = 1.0 / float(np.sqrt(D))
P = 128
NQ = 512                # q window
NW = S // NQ            # 4 windows
WS = 64.0               # weight pre-scale (wqkv, wo)
VS = 0.25               # v-natural extra scale -> v carries 64*0.25 = 16
OUTSCALE = 1.0 / (WS * WS * VS)   # host-side final unscale (1/1024)
EXPSCALE = SCALE / (WS * WS)
BF = mybir.dt.bfloat16
F8 = mybir.dt.float8e4
F32 = mybir.dt.float32
DR = mybir.MatmulPerfMode.DoubleRow
E4 = ml_dtypes.float8_e4m3
BF_NP = ml_dtypes.bfloat16

_CACHE = {}


def _build():
    nc = bacc.Bacc("TRN2", target_bir_lowering=False, debug=False, num_devices=8)
    xhiT = nc.dram_tensor("xhiT", [E, S], F8, kind="ExternalInput").ap()
    xloT = nc.dram_tensor("xloT", [E, S], F8, kind="ExternalInput").ap()
    wmain = nc.dram_tensor("wmain", [E, 2, 768], F8, kind="ExternalInput").ap()
    womain = nc.dram_tensor("womain", [512, 2, E], F8, kind="ExternalInput").ap()
    identq = nc.dram_tensor("identq", [P, P], BF, kind="ExternalInput").ap()
    tri = nc.dram_tensor("tri", [P, P], BF, kind="ExternalInput").ap()
    outT = nc.dram_tensor("outT", [E, S], BF, kind="ExternalOutput").ap()

    EK = E // P          # 16 contraction chunks
    with tile.TileContext(nc) as tc:
        with tc.tile_pool(name="persist", bufs=1) as pp, \
             tc.tile_pool(name="probs", bufs=12) as prb, \
             tc.tile_pool(name="sacc", bufs=4) as sap, \
             tc.tile_pool(name="attn", bufs=2) as atp, \
             tc.tile_pool(name="bcast", bufs=2) as bcp, \
             tc.tile_pool(name="small", bufs=2) as smp, \
             tc.tile_pool(name="outp", bufs=6) as outp, \
             tc.tile_pool(name="ps_proj", bufs=2, space="PSUM") as ps_proj, \
             tc.tile_pool(name="ps_s", bufs=2, space="PSUM") as ps_sp, \
             tc.tile_pool(name="ps_o", bufs=2, space="PSUM") as ps_op:

            # ---- persistent SBUF tiles ----
            tri_sb = pp.tile([P, P], BF, tag="tri", name="tri")
            id_sb = pp.tile([P, P], BF, tag="identq", name="identq")
            ones_sb = pp.tile([P, P], BF, tag="ones", name="ones")
            nc.gpsimd.memset(ones_sb, 1.0)

            # Weight/x storage as single big tiles so loads batch into few
            # DMA instructions.  wmain interleaves (hi,lo) per chunk so each
            # arriving chunk-pair DMA unlocks its full 3-term DR group.
            # x loads issue on the Activation HWDGE queue in parallel with
            # the SP queue's weight loads (~650ns serial issue per queue).
            # x is consumed window-by-window: double-buffer the two live
            # windows instead of holding all of S (frees 32KB/partition)
            wm_sb = pp.tile([P, EK, 2, 768], F8, tag="wm", name="wm")
            xhi_sb = pp.tile([P, EK, 2, NQ], F8, tag="xh", name="xh")
            xlo_sb = pp.tile([P, EK // 2, 2, 2, NQ], F8, tag="xl", name="xl")
            wom_sb = pp.tile([P, 4, 2, E], F8, tag="wom", name="wom")
            # wm on the SP queue, x window-0 on the Activation HWDGE queue:
            # two parallel ~650ns/DMA issue streams; the shared DMA bus then
            # delivers chunk pairs in consumption order
            for g in range(8):
                nc.sync.dma_start(
                    out=wm_sb[:, 2 * g:2 * g + 2].rearrange(
                        "p k two c -> p k (two c)"),
                    in_=wmain[g * 256:(g + 1) * 256].rearrange(
                        "(k p) two c -> p k (two c)", p=P))
                nc.scalar.dma_start(
                    out=xhi_sb[:, 2 * g:2 * g + 2, 0],
                    in_=xhiT[g * 256:(g + 1) * 256, 0:NQ].rearrange(
                        "(k p) c -> p k c", p=P))
                if g % 4 == 1:
                    gg = g // 4
                    nc.scalar.dma_start(
                        out=xlo_sb[:, 4 * gg:4 * gg + 4, :, 0].rearrange(
                            "p k two c -> p (k two) c"),
                        in_=xloT[gg * 1024:(gg + 1) * 1024, 0:NQ].rearrange(
                            "(g2 p) c -> p g2 c", p=P))

            nc.scalar.dma_start(out=tri_sb, in_=tri)
            nc.scalar.dma_start(out=id_sb, in_=identq)

            def load_wo():
                nc.sync.dma_start(
                    out=wom_sb.rearrange("p k two c -> p k (two c)"),
                    in_=womain.rearrange("(k p) two c -> p k (two c)", p=P))

            qkv_sb = [pp.tile([P, S], BF, tag=f"qkv{m}", name=f"qkv{m}") for m in range(6)]
            kT = qkv_sb[4]
            vT = qkv_sb[5]
            vn_sb = [pp.tile([P, D], BF, tag=f"vn{kc}", name=f"vn{kc}") for kc in range(S // P)]

            def load_x_window(w):
                sl = slice(w * NQ, (w + 1) * NQ)
                sb = w % 2
                for g in range(2):
                    nc.sync.dma_start(
                        out=xhi_sb[:, 8 * g:8 * g + 8, sb],
                        in_=xhiT[g * 1024:(g + 1) * 1024, sl].rearrange(
                            "(k p) c -> p k c", p=P))
                nc.sync.dma_start(
                    out=xlo_sb[:, :, :, sb].rearrange(
                        "p k two c -> p (k two) c"),
                    in_=xloT[:, sl].rearrange("(g2 p) c -> p g2 c", p=P))

            def proj_window(w, m):
                sl = slice(w * NQ, (w + 1) * NQ)
                sb = w % 2
                ps = ps_proj.tile([P, NQ], F32, tag="proj", name="proj")
                for ke in range(EK):
                    mv = xhi_sb[:, ke, sb].unsqueeze(1).broadcast_to(
                        [P, 2, NQ])
                    nc.tensor.matmul(ps, wm_sb[:, ke, :, m * P:(m + 1) * P],
                                     mv, start=(ke == 0), stop=False,
                                     perf_mode=DR)
                for k in range(EK // 2):
                    nc.tensor.matmul(
                        ps, wm_sb[:, 2 * k:2 * k + 2, 0, m * P:(m + 1) * P],
                        xlo_sb[:, k, :, sb], start=False,
                        stop=(k == EK // 2 - 1), perf_mode=DR)
                if m == 5:     # v gets its natural scale folded in here
                    nc.vector.tensor_scalar_mul(qkv_sb[m][:, sl], ps, VS)
                else:
                    nc.vector.tensor_copy(qkv_sb[m][:, sl], ps)

            def vnat_window(w):
                # transpose vT 128-chunks into vn via the DMA XBAR (frees
                # the PE transpose + DVE psum-drain the old path needed)
                for j in range(4):
                    kc = 4 * w + j
                    nc.sync.dma_start_transpose(
                        vn_sb[kc], vT[:, kc * P:(kc + 1) * P])

            _ot_pend = {}

            def outproj_chunkW(qs, W, me, ahi, alo, single=False):
                sl = slice(qs, qs + W)
                ps = ps_proj.tile([P, NQ], F32, tag="proj", name="proj")
                pw = ps[:, 0:W]
                for j in range(4):
                    mv = ahi[j][:, 0:W].unsqueeze(1).broadcast_to([P, 2, W])
                    nc.tensor.matmul(pw, wom_sb[:, j, :, me * P:(me + 1) * P],
                                     mv, start=(j == 0), stop=False,
                                     perf_mode=DR)
                for p_ in range(2):
                    nc.tensor.matmul(
                        pw, wom_sb[:, 2 * p_:2 * p_ + 2, 0, me * P:(me + 1) * P],
                        alo[p_][:, :, 0:W], start=False, stop=(p_ == 1),
                        perf_mode=DR)
                if single:
                    # tail chunks: own tile + own DMA so the last transfer
                    # is half-size and departs as soon as its copy lands
                    ot = outp.tile([P, NQ], BF, tag="outs", name="outs")
                    [nc.vector.tensor_copy, nc.scalar.copy][me % 2](
                        ot[:, 0:W], pw)
                    nc.sync.dma_start(out=outT[me * P:(me + 1) * P, sl],
                                      in_=ot[:, 0:W])
                    return
                # pairs of me chunks share one SBUF tile and one output DMA
                if me % 2 == 0:
                    _ot_pend["t"] = outp.tile([P, 2, NQ], BF, tag="out",
                                              name="out")
                ot = _ot_pend["t"]
                [nc.vector.tensor_copy, nc.scalar.copy][(me // 2) % 2](
                    ot[:, me % 2, 0:W], pw)
                if me % 2 == 1:
                    nc.sync.dma_start(
                        out=outT[(me - 1) * P:(me + 1) * P, sl].rearrange(
                            "(k p) c -> p k c", p=P),
                        in_=ot[:, :, 0:W])

            def outproj_chunk(w, me, ahi, alo, single=False):
                outproj_chunkW(w * NQ, NQ, me, ahi, alo, single)

            def outproj_pair256(qs, me0, ahi, alo):
                """Two 256-wide out-proj row-chunks (me0, me0+1) side by
                side in ONE PSUM bank: one drain copy + one DMA per pair,
                so the stream stays PE-bound instead of copy-bound."""
                sl = slice(qs, qs + 256)
                ps = ps_proj.tile([P, NQ], F32, tag="proj", name="proj")
                for half in (0, 1):
                    me = me0 + half
                    pw = ps[:, half * 256:(half + 1) * 256]
                    for j in range(4):
                        mv = ahi[j][:, 0:256].unsqueeze(1).broadcast_to(
                            [P, 2, 256])
                        nc.tensor.matmul(
                            pw, wom_sb[:, j, :, me * P:(me + 1) * P], mv,
                            start=(half == 0 and j == 0), stop=False,
                            perf_mode=DR)
                    for p_ in range(2):
                        nc.tensor.matmul(
                            pw,
                            wom_sb[:, 2 * p_:2 * p_ + 2, 0,
                                   me * P:(me + 1) * P],
                            alo[p_][:, :, 0:256], start=False,
                            stop=(half == 1 and p_ == 1), perf_mode=DR)
                ot = outp.tile([P, 2, NQ], BF, tag="out", name="out")
                [nc.vector.tensor_copy, nc.scalar.copy][(me0 // 2) % 2](
                    ot[:, :, 0:256].rearrange("p two c -> p (two c)"), ps)
                nc.sync.dma_start(
                    out=outT[me0 * P:(me0 + 2) * P, sl].rearrange(
                        "(k p) c -> p k c", p=P),
                    in_=ot[:, :, 0:256])

            def attn_pair(w, h0, drain, early_free=False, mm_den=False):
                """Attention for heads h0,h0+1 interleaved per k-block pair.
                Scores/probs live in [P,2,NQ] pair tiles (2 PSUM banks) so
                one exp covers two k-blocks; PE filler thunks (next-window
                projection, previous-window out-proj) are drained between
                pair steps to keep the PE busy while ACT runs exps."""
                qsl0 = w * NQ
                heads = (h0, h0 + 1)
                npairs = 2 * w + 2
                ps_o = {h: ps_op.tile([P, NQ], F32, tag="o", name="o")
                        for h in heads}
                pend = {h: None for h in heads}   # pending group pair-tile
                racc = {h: None for h in heads}
                dtiles = {h: [] for h in heads}

                def flat(ap):
                    return ap.rearrange("p two c -> p (two c)")

                for pi in range(npairs):
                    kc0 = 2 * pi
                    for h in heads:
                        pp_s = ps_sp.tile([P, 2, NQ], F32, tag="s", name="s")
                        pr = prb.tile([P, 2, NQ], BF, tag="pr", name="pr")
                        fps, fpr = flat(pp_s), flat(pr)
                        if pi < 2 * w:          # two full-width blocks
                            for i in range(2):
                                nc.tensor.matmul(
                                    pp_s[:, i, :],
                                    kT[:, (kc0 + i) * P:(kc0 + i + 1) * P],
                                    qkv_sb[h][:, qsl0:qsl0 + NQ],
                                    start=True, stop=True)
                            nc.scalar.activation(
                                fpr, fps, mybir.ActivationFunctionType.Exp,
                                scale=EXPSCALE)
                            for i in range(2):
                                nc.tensor.matmul(ps_o[h], vn_sb[kc0 + i],
                                                 pr[:, i, :],
                                                 start=(kc0 + i == 0),
                                                 stop=False,
                                                 skip_group_check=True)
                            if pend[h] is None:
                                pend[h] = pr
                            else:               # fold group of 4 into racc
                                sa = sap.tile([P, 2, NQ], BF, tag="sa",
                                              name="sa")
                                nc.vector.tensor_add(sa, flat(pend[h]), fpr)
                                if racc[h] is None:
                                    racc[h] = sa
                                else:
                                    nc.vector.tensor_add(
                                        flat(racc[h]), flat(racc[h]),
                                        flat(sa))
                                pend[h] = None
                        elif pi == 2 * w:       # diag D1: widths 512, 384
                            nc.tensor.matmul(
                                pp_s[:, 0, :], kT[:, kc0 * P:(kc0 + 1) * P],
                                qkv_sb[h][:, qsl0:qsl0 + NQ],
                                start=True, stop=True)
                            nc.tensor.matmul(
                                pp_s[:, 1, 0:384],
                                kT[:, (kc0 + 1) * P:(kc0 + 2) * P],
                                qkv_sb[h][:, qsl0 + P:qsl0 + NQ],
                                start=True, stop=True)
                            nc.scalar.activation(
                                fpr[:, 0:896], fps[:, 0:896],
                                mybir.ActivationFunctionType.Exp,
                                scale=EXPSCALE)
                            for i in range(2):
                                nc.vector.tensor_mul(pr[:, i, 0:P],
                                                     pr[:, i, 0:P], tri_sb)
                            nc.tensor.matmul(ps_o[h], vn_sb[kc0], pr[:, 0, :],
                                             start=(kc0 == 0), stop=False,
                                             skip_group_check=True)
                            nc.tensor.matmul(ps_o[h][:, P:], vn_sb[kc0 + 1],
                                             pr[:, 1, 0:384],
                                             start=False, stop=False,
                                             skip_group_check=True)
                            dtiles[h].append(pr)
                        else:                   # diag D2: widths 256, 128
                            nc.tensor.matmul(
                                pp_s[:, 0, 0:256],
                                kT[:, kc0 * P:(kc0 + 1) * P],
                                qkv_sb[h][:, qsl0 + 2 * P:qsl0 + NQ],
                                start=True, stop=True)
                            nc.tensor.matmul(
                                pp_s[:, 1, 0:P],
                                kT[:, (kc0 + 1) * P:(kc0 + 2) * P],
                                qkv_sb[h][:, qsl0 + 3 * P:qsl0 + NQ],
                                start=True, stop=True)
                            nc.scalar.activation(
                                pr[:, 0, 0:256], pp_s[:, 0, 0:256],
                                mybir.ActivationFunctionType.Exp,
                                scale=EXPSCALE)
                            nc.scalar.activation(
                                pr[:, 1, 0:P], pp_s[:, 1, 0:P],
                                mybir.ActivationFunctionType.Exp,
                                scale=EXPSCALE)
                            for i in range(2):
                                nc.vector.tensor_mul(pr[:, i, 0:P],
                                                     pr[:, i, 0:P], tri_sb)
                            nc.tensor.matmul(ps_o[h][:, 2 * P:],
                                             vn_sb[kc0], pr[:, 0, 0:256],
                                             start=False, stop=False,
                                             skip_group_check=True)
                            nc.tensor.matmul(ps_o[h][:, 3 * P:],
                                             vn_sb[kc0 + 1], pr[:, 1, 0:P],
                                             start=False, stop=True,
                                             skip_group_check=True)
                            dtiles[h].append(pr)
                    drain()

                out = []
                # last-window pair 0: drain PV to SBUF immediately so ps_o
                # recycles for pair 1 without waiting the normalize chain
                tn = {}
                if early_free:
                    for h in heads:
                        tn[h] = atp.tile([P, NQ], F32, tag=f"tn{h % 2}",
                                         name=f"tn{h % 2}")
                        nc.vector.tensor_copy(tn[h], ps_o[h])
                for h in heads:
                    d1, d2 = dtiles[h]
                    da = d1[:, 0, :]            # diag accum, in place
                    nc.vector.tensor_add(da[:, P:], da[:, P:],
                                         flat(d1)[:, NQ:NQ + 384])
                    nc.vector.tensor_add(da[:, 2 * P:], da[:, 2 * P:],
                                         d2[:, 0, 0:256])
                    nc.vector.tensor_add(da[:, 3 * P:], da[:, 3 * P:],
                                         d2[:, 1, 0:P])
                    if racc[h] is None:
                        acc = da
                    else:
                        fa = sap.tile([P, NQ], BF, tag="fa", name="fa")
                        nc.vector.tensor_add(fa, racc[h][:, 0, :],
                                             racc[h][:, 1, :])
                        nc.vector.tensor_add(fa, fa, da)
                        acc = fa
                    if mm_den:
                        # denominator as a ones-matmul: every PSUM row gets
                        # sum-over-partitions(acc).  Cheap on PE (512 cy) and
                        # drops the slow Pool all-reduce from the last
                        # window's tail-critical normalize chain.  Borrow a
                        # scores-pool tile (idle once attention is done) so
                        # ps_proj stays free for the final out-proj chunks.
                        den_t = ps_sp.tile([P, 2, NQ], F32, tag="s",
                                           name="s")
                        den = den_t[:, 0, :]
                        nc.tensor.matmul(den, ones_sb, acc, start=True,
                                         stop=True)
                    else:
                        den = bcp.tile([P, NQ], F32, tag="ar", name="ar")
                        nc.gpsimd.partition_all_reduce(den, acc, P,
                                                       bass_isa.ReduceOp.add)
                    bc = bcp.tile([P, NQ], F32, tag="bc", name="bc")
                    nc.vector.reciprocal(bc, den)
                    t = atp.tile([P, NQ], BF, tag=f"t{h % 2}",
                                 name=f"t{h % 2}")
                    nc.vector.tensor_mul(t, tn[h] if early_free else ps_o[h],
                                         bc)
                    ahi = atp.tile([P, NQ], F8, tag=f"ahi{h}", name=f"ahi{h}")
                    nc.scalar.copy(ahi, t)
                    out.append((t, ahi))
                    drain()
                return out

            def attn_half(qs, h0, drain, mm_den=False):
                """256-wide attention half-window for heads h0,h0+1: four
                256-col k-blocks pack into one [P,2,NQ] pair tile (one exp
                per 4 blocks).  Used to split the LAST window so all tail
                ops (normalize chain, out-proj, final DMA) halve, and the
                first half's out-proj overlaps the second half."""
                W2 = 256
                heads = (h0, h0 + 1)
                nf = qs // P                 # full k-blocks before the diag
                ps_o = {h: ps_op.tile([P, NQ], F32, tag="o", name="o")
                        for h in heads}
                packed = {h: [] for h in heads}
                dtile = {}

                def flat(ap):
                    return ap.rearrange("p two c -> p (two c)")

                for s in range(0, nf, 4):
                    e = min(s + 4, nf)
                    for h in heads:
                        pp_s = ps_sp.tile([P, 2, NQ], F32, tag="s", name="s")
                        pr = prb.tile([P, 2, NQ], BF, tag="pr", name="pr")
                        for j, kc in enumerate(range(s, e)):
                            pl, off = j // 2, (j % 2) * W2
                            nc.tensor.matmul(
                                pp_s[:, pl, off:off + W2],
                                kT[:, kc * P:(kc + 1) * P],
                                qkv_sb[h][:, qs:qs + W2],
                                start=True, stop=True)
                        nn = (e - s) * W2
                        nc.scalar.activation(
                            flat(pr)[:, 0:nn], flat(pp_s)[:, 0:nn],
                            mybir.ActivationFunctionType.Exp, scale=EXPSCALE)
                        for j, kc in enumerate(range(s, e)):
                            pl, off = j // 2, (j % 2) * W2
                            nc.tensor.matmul(ps_o[h][:, 0:W2], vn_sb[kc],
                                             pr[:, pl, off:off + W2],
                                             start=(kc == 0), stop=False,
                                             skip_group_check=True)
                        packed[h].append((pr, e - s))
                    drain()
                for h in heads:          # diag pair: widths 256, 128
                    pp_s = ps_sp.tile([P, 2, NQ], F32, tag="s", name="s")
                    pr = prb.tile([P, 2, NQ], BF, tag="pr", name="pr")
                    nc.tensor.matmul(pp_s[:, 0, 0:W2],
                                     kT[:, nf * P:(nf + 1) * P],
                                     qkv_sb[h][:, qs:qs + W2],
                                     start=True, stop=True)
                    nc.tensor.matmul(pp_s[:, 1, 0:P],
                                     kT[:, (nf + 1) * P:(nf + 2) * P],
                                     qkv_sb[h][:, qs + P:qs + W2],
                                     start=True, stop=True)
                    nc.scalar.activation(pr[:, 0, 0:W2], pp_s[:, 0, 0:W2],
                                         mybir.ActivationFunctionType.Exp,
                                         scale=EXPSCALE)
                    nc.scalar.activation(pr[:, 1, 0:P], pp_s[:, 1, 0:P],
                                         mybir.ActivationFunctionType.Exp,
                                         scale=EXPSCALE)
                    for i in range(2):
                        nc.vector.tensor_mul(pr[:, i, 0:P], pr[:, i, 0:P],
                                             tri_sb)
                    nc.tensor.matmul(ps_o[h][:, 0:W2], vn_sb[nf],
                                     pr[:, 0, 0:W2], start=False, stop=False,
                                     skip_group_check=True)
                    nc.tensor.matmul(ps_o[h][:, P:W2], vn_sb[nf + 1],
                                     pr[:, 1, 0:P], start=False, stop=True,
                                     skip_group_check=True)
                    dtile[h] = pr
                    drain()
                out = []
                for h in heads:
                    ft = [flat(pr) for (pr, nb) in packed[h] if nb == 4]
                    pt = [flat(pr)[:, 0:2 * W2]
                          for (pr, nb) in packed[h] if nb == 2]
                    sa = sap.tile([P, 2, NQ], BF, tag="sa", name="sa")
                    fs = flat(sa)
                    nc.vector.tensor_add(fs, ft[0], ft[1])
                    for x in ft[2:]:
                        nc.vector.tensor_add(fs, fs, x)
                    for x in pt:
                        nc.vector.tensor_add(fs[:, 0:2 * W2],
                                             fs[:, 0:2 * W2], x)
                    d = dtile[h]
                    da = d[:, 0, :]
                    nc.vector.tensor_add(da[:, P:W2], da[:, P:W2],
                                         d[:, 1, 0:P])
                    fa = sap.tile([P, NQ], BF, tag="fa", name="fa")
                    fw = fa[:, 0:W2]
                    nc.vector.tensor_add(fw, sa[:, 0, 0:W2],
                                         sa[:, 0, W2:2 * W2])
                    nc.vector.tensor_add(fw, fw, sa[:, 1, 0:W2])
                    nc.vector.tensor_add(fw, fw, sa[:, 1, W2:2 * W2])
                    nc.vector.tensor_add(fw, fw, da[:, 0:W2])
                    if mm_den:
                        den_t = ps_sp.tile([P, 2, NQ], F32, tag="s",
                                           name="s")
                        den = den_t[:, 0, 0:W2]
                        nc.tensor.matmul(den, ones_sb, fw, start=True,
                                         stop=True)
                    else:
                        den_f = bcp.tile([P, NQ], F32, tag="ar", name="ar")
                        den = den_f[:, 0:W2]
                        nc.gpsimd.partition_all_reduce(den, fw, P,
                                                       bass_isa.ReduceOp.add)
                    bc = bcp.tile([P, NQ], F32, tag="bc", name="bc")
                    nc.vector.reciprocal(bc[:, 0:W2], den)
                    t = atp.tile([P, NQ], BF, tag=f"t{h % 2}",
                                 name=f"t{h % 2}")
                    nc.vector.tensor_mul(t[:, 0:W2], ps_o[h][:, 0:W2],
                                         bc[:, 0:W2])
                    ahi = atp.tile([P, NQ], F8, tag=f"ahi{h}", name=f"ahi{h}")
                    nc.scalar.copy(ahi[:, 0:W2], t[:, 0:W2])
                    out.append((t, ahi))
                    drain()
                return out

            # ---------------- main window loop ----------------
            from collections import deque

            def make_drain(fillers, nsteps, boost=1.0):
                st = {"a": 0.0, "r": len(fillers) / nsteps * boost}

                def drain():
                    st["a"] += st["r"]
                    while fillers and st["a"] >= 1.0:
                        st["a"] -= 1.0
                        fillers.popleft()()
                return drain

            for m in (0, 1, 4, 5, 2, 3):
                proj_window(0, m)
            vnat_window(0)
            prev = None       # (ahi list, alo list) of previous window
            for w in range(NW):
                if w + 1 < NW:
                    load_x_window(w + 1)
                if w == 0:
                    load_wo()
                fillers = deque()
                if prev is not None:
                    pv = prev
                    for me in range(16):
                        fillers.append(
                            lambda me=me, pv=pv: outproj_chunk(
                                w - 1, me, pv[0], pv[1]))
                if w + 1 < NW:
                    for m in (0, 1, 4, 5, 2, 3):
                        fillers.append(
                            lambda m=m, w1=w + 1: proj_window(w1, m))
                    fillers.append(lambda w1=w + 1: vnat_window(w1))
                nsteps = 2 * (2 * w + 2) + 4
                last = w == NW - 1
                drain = make_drain(fillers, nsteps, boost=1.0)

                def finish_heads(res, W):
                    ahi_l, alo_l = [], [None, None]
                    for h, (t, ahi) in enumerate(res):
                        if h % 2 == 0:
                            alo_l[h // 2] = atp.tile(
                                [P, 2, NQ], F8, tag=f"alo{h // 2}",
                                name=f"alo{h // 2}")
                        nc.vector.scalar_tensor_tensor(
                            alo_l[h // 2][:, h % 2, 0:W], t[:, 0:W], 1.0,
                            ahi[:, 0:W],
                            mybir.AluOpType.mult, mybir.AluOpType.subtract)
                        ahi_l.append(ahi)
                    return ahi_l, alo_l

                if not last:
                    res = []
                    for h0 in (0, 2):
                        res += attn_pair(w, h0, drain)
                    prev = finish_heads(res, NQ)
                else:
                    # last window: two 256-wide halves.  The A half's
                    # out-proj becomes PE filler work for the B half, and
                    # the tail-critical normalize/out-proj/DMA all halve.
                    resA = []
                    for h0 in (0, 2):
                        resA += attn_half(S - 2 * 256, h0, drain)
                    ahiA, aloA = finish_heads(resA, 256)
                    for me0 in range(0, 16, 2):
                        fillers.append(
                            lambda me0=me0: outproj_pair256(
                                S - 2 * 256, me0, ahiA, aloA))
                    resB = []
                    for h0 in (0, 2):
                        resB += attn_half(S - 256, h0, drain, mm_den=True)
                    prev = finish_heads(resB, 256)
                while fillers:
                    fillers.popleft()()
            for me0 in range(0, 14, 2):
                outproj_pair256(S - 256, me0, prev[0], prev[1])
            for me in (14, 15):
                outproj_chunkW(S - 256, 256, me, prev[0], prev[1],
                               single=True)

    nc.finalize()
    # The standard compile pipeline leaves the (unused) register preamble
    # in place here, which the walrus birverifier then rejects with "Reg
    # has not been allocated yet"; a second DCE pass removes it.
    nc.dce_regs()
    return nc


def _get_nc():
    if "nc" not in _CACHE:
        _CACHE["nc"] = _build()
    return _CACHE["nc"]


def _hi_lo(a):
    hi = a.astype(E4)
    lo = (a - hi.astype(np.float32)).astype(E4)
    return hi, lo


def kernel(x, Wq, Wk, Wv, Wo, _trace=False, _tmpdir=None):
    x = np.asarray(x, np.float32)
    Wq, Wk, Wv, Wo = (np.asarray(a, np.float32) for a in (Wq, Wk, Wv, Wo))
    nc = _get_nc()
    identq = np.eye(P, dtype=np.float32).astype(BF_NP)
    tri = np.triu(np.ones((P, P), np.float32)).astype(BF_NP)
    from concurrent.futures import ThreadPoolExecutor

    def _xprep(b):
        xT = np.ascontiguousarray(x[b].T)
        return _hi_lo(xT)

    with ThreadPoolExecutor(8) as tp:
        xp = list(tp.map(_xprep, range(B)))

        def _core(c):
            b, g = c // 4, c % 4
            wqkv = np.concatenate(
                [Wq[512 * g:512 * (g + 1)],
                 Wk[128 * g:128 * (g + 1)],
                 Wv[128 * g:128 * (g + 1)]], axis=0) * WS
            wqkvT = np.ascontiguousarray(wqkv.T)
            whi, wlo = _hi_lo(wqkvT)
            wmain = np.ascontiguousarray(np.stack([whi, wlo], axis=1))
            woT = np.ascontiguousarray(Wo[:, 512 * g:512 * (g + 1)].T) * WS
            wohi, wolo = _hi_lo(woT)
            womain = np.ascontiguousarray(np.stack([wohi, wolo], axis=1))
            return {
                "xhiT": xp[b][0], "xloT": xp[b][1],
                "wmain": wmain, "womain": womain,
                "identq": identq, "tri": tri,
            }
        in_maps = list(tp.map(_core, range(8)))
    res = bass_utils.run_bass_kernel_spmd(
        nc, in_maps, core_ids=list(range(8)), trace=_trace, tmpdir=_tmpdir)
    out = np.zeros((B, S, E), np.float32)
    for c in range(8):
        out[c // 4] += res.results[c]["outT"].astype(np.float32).T
    out *= OUTSCALE
    if _trace:
        return out, res
    return out



# revision 57
# speedup vs baseline: 1.0113x; 1.0089x over previous
"""Grouped-Query Attention (B=2, S=2048, E=2048, H=16, KVH=4, D=128, causal)
as a Bass/Tile kernel on 8 Trainium2 NeuronCores.

Sharding: core c handles batch b=c//4 and kv-head-group g=c%4 (4 q heads +
1 kv head per core).  Out-proj is row-sharded: each core computes a partial
[E,S] (transposed, x1024-scaled bf16) output; host scales+sums the 4
partials per batch.

Key speed tricks vs the bf16 baseline:
 * Projections and out-proj run as fp8e4m3 DoubleRow matmuls (0.5 cy/row,
   2 contraction chunks per instruction).  Each operand is split hi/lo
   (A = fp8(A) + fp8(A - fp8(A))); main term pairs (W_hi,W_lo)x(x_hi,x_hi)
   [stride-0 broadcast], correction term pairs chunk pairs of W_hi x x_lo.
   Dropping the lo*lo term leaves ~2^-8 relative error at 0.75x the bf16
   PE cost.  Weights are pre-scaled by 64 so the fp8-lo parts stay normal.
 * Causal diagonal 512x512 super-blocks use shrinking-width score/exp/PV
   ops (512/384/256/128) instead of full width + masking.
 * Softmax denominator: probs tiles are tree-added on the DVE (bf16);
   windows 0-2 reduce across partitions on GPSIMD, the last window uses a
   ones-matmul on the PE instead (drops the slow Pool all-reduce from the
   tail-critical normalize chain; its PSUM tile borrows the then-idle
   scores pool so the final out-proj keeps both ps_proj banks).
 * v-transpose (vn tiles for PV) via the DMA XBAR transpose instead of PE
   identity-matmuls + DVE PSUM drains; the v natural scale folds into the
   PSUM->SBUF copy (tensor_scalar_mul).
 * Per-window loop (512 q columns) interleaves projection, attention and
   the PREVIOUS window's out-proj so the PE stays fed while the ACT engine
   computes exps; x is DMA'd window-by-window into a 2-window double
   buffer (frees 32KB/partition of SBUF); startup x loads issue on the
   Activation HWDGE queue in parallel with SP's weight loads.
 * The LAST window's attention/out-proj splits into two 256-wide halves
   (attn_half: four 256-col k-blocks pack per [P,2,NQ] pair tile, one exp
   per 4 blocks).  The first half's out-proj overlaps the second half's
   attention, and the tail-critical normalize chain, final out-proj and
   last DMA all halve.  256-wide out-proj row-chunks go two-per-PSUM-bank
   (outproj_pair256) so the final stream stays PE-bound, and the last 2
   chunks get their own half-size DMAs so the tail transfer departs early.
"""
import sys

for _p in ("/opt/trn_rl_repo", "/root/.axon_site/_ro/trn_rl_repo"):
    if _p not in sys.path:
        sys.path.append(_p)

import numpy as np
import ml_dtypes

import concourse.bass as bass
import concourse.bass_isa as bass_isa
import concourse.mybir as mybir
import concourse.tile as tile
from concourse import bacc, bass_utils

B, S, E = 2, 2048, 2048
H, KVH = 16, 4
D = E // H              # 128
HPC = 4                 # q heads per core
SCALE = 1.0 / float(np.sqrt(D))
P = 128
NQ = 512                # q window
NW = S // NQ            # 4 windows
WS = 64.0               # weight pre-scale (wqkv, wo)
VS = 0.25               # v-natural extra scale -> v carries 64*0.25 = 16
OUTSCALE = 1.0 / (WS * WS * VS)   # host-side final unscale (1/1024)
EXPSCALE = SCALE / (WS * WS)
BF = mybir.dt.bfloat16
F8 = mybir.dt.float8e4
F32 = mybir.dt.float32
DR = mybir.MatmulPerfMode.DoubleRow
E4 = ml_dtypes.float8_e4m3
BF_NP = ml_dtypes.bfloat16

_CACHE = {}


def _build():
    nc = bacc.Bacc("TRN2", target_bir_lowering=False, debug=False, num_devices=8)
    xhiT = nc.dram_tensor("xhiT", [E, S], F8, kind="ExternalInput").ap()
    xloT = nc.dram_tensor("xloT", [E, S], F8, kind="ExternalInput").ap()
    wmain = nc.dram_tensor("wmain", [E, 2, 768], F8, kind="ExternalInput").ap()
    womain = nc.dram_tensor("womain", [512, 2, E], F8, kind="ExternalInput").ap()
    identq = nc.dram_tensor("identq", [P, P], BF, kind="ExternalInput").ap()
    tri = nc.dram_tensor("tri", [P, P], BF, kind="ExternalInput").ap()
    outT = nc.dram_tensor("outT", [E, S], BF, kind="ExternalOutput").ap()

    EK = E // P          # 16 contraction chunks
    with tile.TileContext(nc) as tc:
        with tc.tile_pool(name="persist", bufs=1) as pp, \
             tc.tile_pool(name="probs", bufs=12) as prb, \
             tc.tile_pool(name="sacc", bufs=4) as sap, \
             tc.tile_pool(name="attn", bufs=2) as atp, \
             tc.tile_pool(name="bcast", bufs=2) as bcp, \
             tc.tile_pool(name="small", bufs=2) as smp, \
             tc.tile_pool(name="outp", bufs=6) as outp, \
             tc.tile_pool(name="ps_proj", bufs=2, space="PSUM") as ps_proj, \
             tc.tile_pool(name="ps_s", bufs=2, space="PSUM") as ps_sp, \
             tc.tile_pool(name="ps_o", bufs=2, space="PSUM") as ps_op:

            # ---- persistent SBUF tiles ----
            tri_sb = pp.tile([P, P], BF, tag="tri", name="tri")
            id_sb = pp.tile([P, P], BF, tag="identq", name="identq")
            ones_sb = pp.tile([P, P], BF, tag="ones", name="ones")
            nc.gpsimd.memset(ones_sb, 1.0)

            # Weight/x storage as single big tiles so loads batch into few
            # DMA instructions.  wmain interleaves (hi,lo) per chunk so each
            # arriving chunk-pair DMA unlocks its full 3-term DR group.
            # x loads issue on the Activation HWDGE queue in parallel with
            # the SP queue's weight loads (~650ns serial issue per queue).
            # x is consumed window-by-window: double-buffer the two live
            # windows instead of holding all of S (frees 32KB/partition)
            wm_sb = pp.tile([P, EK, 2, 768], F8, tag="wm", name="wm")
            xhi_sb = pp.tile([P, EK, 2, NQ], F8, tag="xh", name="xh")
            xlo_sb = pp.tile([P, EK // 2, 2, 2, NQ], F8, tag="xl", name="xl")
            wom_sb = pp.tile([P, 4, 2, E], F8, tag="wom", name="wom")
            # wm on the SP queue, x window-0 on the Activation HWDGE queue:
            # two parallel ~650ns/DMA issue streams; the shared DMA bus then
            # delivers chunk pairs in consumption order
            for g in range(8):
                wq = nc.sync
                xq = nc.scalar
                wq.dma_start(
                    out=wm_sb[:, 2 * g:2 * g + 2].rearrange(
                        "p k two c -> p k (two c)"),
                    in_=wmain[g * 256:(g + 1) * 256].rearrange(
                        "(k p) two c -> p k (two c)", p=P))
                xq.dma_start(
                    out=xhi_sb[:, 2 * g:2 * g + 2, 0],
                    in_=xhiT[g * 256:(g + 1) * 256, 0:NQ].rearrange(
                        "(k p) c -> p k c", p=P))
                if g % 4 == 1:
                    gg = g // 4
                    nc.scalar.dma_start(
                        out=xlo_sb[:, 4 * gg:4 * gg + 4, :, 0].rearrange(
                            "p k two c -> p (k two) c"),
                        in_=xloT[gg * 1024:(gg + 1) * 1024, 0:NQ].rearrange(
                            "(g2 p) c -> p g2 c", p=P))

            nc.scalar.dma_start(out=tri_sb, in_=tri)
            nc.scalar.dma_start(out=id_sb, in_=identq)

            def load_wo():
                nc.sync.dma_start(
                    out=wom_sb.rearrange("p k two c -> p k (two c)"),
                    in_=womain.rearrange("(k p) two c -> p k (two c)", p=P))

            qkv_sb = [pp.tile([P, S], BF, tag=f"qkv{m}", name=f"qkv{m}") for m in range(6)]
            kT = qkv_sb[4]
            vT = qkv_sb[5]
            vn_sb = [pp.tile([P, D], BF, tag=f"vn{kc}", name=f"vn{kc}") for kc in range(S // P)]

            def load_x_window(w):
                sl = slice(w * NQ, (w + 1) * NQ)
                sb = w % 2
                for g in range(2):
                    nc.sync.dma_start(
                        out=xhi_sb[:, 8 * g:8 * g + 8, sb],
                        in_=xhiT[g * 1024:(g + 1) * 1024, sl].rearrange(
                            "(k p) c -> p k c", p=P))
                nc.sync.dma_start(
                    out=xlo_sb[:, :, :, sb].rearrange(
                        "p k two c -> p (k two) c"),
                    in_=xloT[:, sl].rearrange("(g2 p) c -> p g2 c", p=P))

            def proj_window(w, m):
                sl = slice(w * NQ, (w + 1) * NQ)
                sb = w % 2
                ps = ps_proj.tile([P, NQ], F32, tag="proj", name="proj")
                for ke in range(EK):
                    mv = xhi_sb[:, ke, sb].unsqueeze(1).broadcast_to(
                        [P, 2, NQ])
                    nc.tensor.matmul(ps, wm_sb[:, ke, :, m * P:(m + 1) * P],
                                     mv, start=(ke == 0), stop=False,
                                     perf_mode=DR)
                for k in range(EK // 2):
                    nc.tensor.matmul(
                        ps, wm_sb[:, 2 * k:2 * k + 2, 0, m * P:(m + 1) * P],
                        xlo_sb[:, k, :, sb], start=False,
                        stop=(k == EK // 2 - 1), perf_mode=DR)
                if m == 5:     # v gets its natural scale folded in here
                    nc.vector.tensor_scalar_mul(qkv_sb[m][:, sl], ps, VS)
                else:
                    nc.vector.tensor_copy(qkv_sb[m][:, sl], ps)

            def vnat_window(w):
                # transpose vT 128-chunks into vn via the DMA XBAR (frees
                # the PE transpose + DVE psum-drain the old path needed)
                for j in range(4):
                    kc = 4 * w + j
                    nc.sync.dma_start_transpose(
                        vn_sb[kc], vT[:, kc * P:(kc + 1) * P])

            _ot_pend = {}

            def outproj_chunkW(qs, W, me, ahi, alo, single=False):
                sl = slice(qs, qs + W)
                ps = ps_proj.tile([P, NQ], F32, tag="proj", name="proj")
                pw = ps[:, 0:W]
                for j in range(4):
                    mv = ahi[j][:, 0:W].unsqueeze(1).broadcast_to([P, 2, W])
                    nc.tensor.matmul(pw, wom_sb[:, j, :, me * P:(me + 1) * P],
                                     mv, start=(j == 0), stop=False,
                                     perf_mode=DR)
                for p_ in range(2):
                    nc.tensor.matmul(
                        pw, wom_sb[:, 2 * p_:2 * p_ + 2, 0, me * P:(me + 1) * P],
                        alo[p_][:, :, 0:W], start=False, stop=(p_ == 1),
                        perf_mode=DR)
                if single:
                    # tail chunks: own tile + own DMA so the last transfer
                    # is half-size and departs as soon as its copy lands
                    ot = outp.tile([P, NQ], BF, tag="outs", name="outs")
                    [nc.vector.tensor_copy, nc.scalar.copy][me % 2](
                        ot[:, 0:W], pw)
                    nc.sync.dma_start(out=outT[me * P:(me + 1) * P, sl],
                                      in_=ot[:, 0:W])
                    return
                # pairs of me chunks share one SBUF tile and one output DMA
                if me % 2 == 0:
                    _ot_pend["t"] = outp.tile([P, 2, NQ], BF, tag="out",
                                              name="out")
                ot = _ot_pend["t"]
                [nc.vector.tensor_copy, nc.scalar.copy][(me // 2) % 2](
                    ot[:, me % 2, 0:W], pw)
                if me % 2 == 1:
                    nc.sync.dma_start(
                        out=outT[(me - 1) * P:(me + 1) * P, sl].rearrange(
                            "(k p) c -> p k c", p=P),
                        in_=ot[:, :, 0:W])

            def outproj_chunk(w, me, ahi, alo, single=False):
                outproj_chunkW(w * NQ, NQ, me, ahi, alo, single)

            def outproj_pair256(qs, me0, ahi, alo):
                """Two 256-wide out-proj row-chunks (me0, me0+1) side by
                side in ONE PSUM bank: one drain copy + one DMA per pair,
                so the stream stays PE-bound instead of copy-bound."""
                sl = slice(qs, qs + 256)
                ps = ps_proj.tile([P, NQ], F32, tag="proj", name="proj")
                for half in (0, 1):
                    me = me0 + half
                    pw = ps[:, half * 256:(half + 1) * 256]
                    for j in range(4):
                        mv = ahi[j][:, 0:256].unsqueeze(1).broadcast_to(
                            [P, 2, 256])
                        nc.tensor.matmul(
                            pw, wom_sb[:, j, :, me * P:(me + 1) * P], mv,
                            start=(half == 0 and j == 0), stop=False,
                            perf_mode=DR)
                    for p_ in range(2):
                        nc.tensor.matmul(
                            pw,
                            wom_sb[:, 2 * p_:2 * p_ + 2, 0,
                                   me * P:(me + 1) * P],
                            alo[p_][:, :, 0:256], start=False,
                            stop=(half == 1 and p_ == 1), perf_mode=DR)
                ot = outp.tile([P, 2, NQ], BF, tag="out", name="out")
                [nc.vector.tensor_copy, nc.scalar.copy][(me0 // 2) % 2](
                    ot[:, :, 0:256],
                    ps.rearrange("p (two c) -> p two c", two=2))
                nc.sync.dma_start(
                    out=outT[me0 * P:(me0 + 2) * P, sl].rearrange(
                        "(k p) c -> p k c", p=P),
                    in_=ot[:, :, 0:256])

            def attn_pair(w, h0, drain, early_free=False, mm_den=False):
                """Attention for heads h0,h0+1 interleaved per k-block pair.
                Scores/probs live in [P,2,NQ] pair tiles (2 PSUM banks) so
                one exp covers two k-blocks; PE filler thunks (next-window
                projection, previous-window out-proj) are drained between
                pair steps to keep the PE busy while ACT runs exps."""
                qsl0 = w * NQ
                heads = (h0, h0 + 1)
                npairs = 2 * w + 2
                ps_o = {h: ps_op.tile([P, NQ], F32, tag="o", name="o")
                        for h in heads}
                pend = {h: None for h in heads}   # pending group pair-tile
                racc = {h: None for h in heads}
                dtiles = {h: [] for h in heads}

                def flat(ap):
                    return ap.rearrange("p two c -> p (two c)")

                for pi in range(npairs):
                    kc0 = 2 * pi
                    for h in heads:
                        pp_s = ps_sp.tile([P, 2, NQ], F32, tag="s", name="s")
                        pr = prb.tile([P, 2, NQ], BF, tag="pr", name="pr")
                        fps, fpr = flat(pp_s), flat(pr)
                        if pi < 2 * w:          # two full-width blocks
                            for i in range(2):
                                nc.tensor.matmul(
                                    pp_s[:, i, :],
                                    kT[:, (kc0 + i) * P:(kc0 + i + 1) * P],
                                    qkv_sb[h][:, qsl0:qsl0 + NQ],
                                    start=True, stop=True)
                            nc.scalar.activation(
                                fpr, fps, mybir.ActivationFunctionType.Exp,
                                scale=EXPSCALE)
                            for i in range(2):
                                nc.tensor.matmul(ps_o[h], vn_sb[kc0 + i],
                                                 pr[:, i, :],
                                                 start=(kc0 + i == 0),
                                                 stop=False,
                                                 skip_group_check=True)
                            if pend[h] is None:
                                pend[h] = pr
                            else:               # fold group of 4 into racc
                                sa = sap.tile([P, 2, NQ], BF, tag="sa",
                                              name="sa")
                                nc.vector.tensor_add(sa, flat(pend[h]), fpr)
                                if racc[h] is None:
                                    racc[h] = sa
                                else:
                                    nc.vector.tensor_add(
                                        flat(racc[h]), flat(racc[h]),
                                        flat(sa))
                                pend[h] = None
                        elif pi == 2 * w:       # diag D1: widths 512, 384
                            nc.tensor.matmul(
                                pp_s[:, 0, :], kT[:, kc0 * P:(kc0 + 1) * P],
                                qkv_sb[h][:, qsl0:qsl0 + NQ],
                                start=True, stop=True)
                            nc.tensor.matmul(
                                pp_s[:, 1, 0:384],
                                kT[:, (kc0 + 1) * P:(kc0 + 2) * P],
                                qkv_sb[h][:, qsl0 + P:qsl0 + NQ],
                                start=True, stop=True)
                            nc.scalar.activation(
                                fpr[:, 0:896], fps[:, 0:896],
                                mybir.ActivationFunctionType.Exp,
                                scale=EXPSCALE)
                            for i in range(2):
                                nc.vector.tensor_mul(pr[:, i, 0:P],
                                                     pr[:, i, 0:P], tri_sb)
                            nc.tensor.matmul(ps_o[h], vn_sb[kc0], pr[:, 0, :],
                                             start=(kc0 == 0), stop=False,
                                             skip_group_check=True)
                            nc.tensor.matmul(ps_o[h][:, P:], vn_sb[kc0 + 1],
                                             pr[:, 1, 0:384],
                                             start=False, stop=False,
                                             skip_group_check=True)
                            dtiles[h].append(pr)
                        else:                   # diag D2: widths 256, 128
                            nc.tensor.matmul(
                                pp_s[:, 0, 0:256],
                                kT[:, kc0 * P:(kc0 + 1) * P],
                                qkv_sb[h][:, qsl0 + 2 * P:qsl0 + NQ],
                                start=True, stop=True)
                            nc.tensor.matmul(
                                pp_s[:, 1, 0:P],
                                kT[:, (kc0 + 1) * P:(kc0 + 2) * P],
                                qkv_sb[h][:, qsl0 + 3 * P:qsl0 + NQ],
                                start=True, stop=True)
                            nc.scalar.activation(
                                pr[:, 0, 0:256], pp_s[:, 0, 0:256],
                                mybir.ActivationFunctionType.Exp,
                                scale=EXPSCALE)
                            nc.scalar.activation(
                                pr[:, 1, 0:P], pp_s[:, 1, 0:P],
                                mybir.ActivationFunctionType.Exp,
                                scale=EXPSCALE)
                            for i in range(2):
                                nc.vector.tensor_mul(pr[:, i, 0:P],
                                                     pr[:, i, 0:P], tri_sb)
                            nc.tensor.matmul(ps_o[h][:, 2 * P:],
                                             vn_sb[kc0], pr[:, 0, 0:256],
                                             start=False, stop=False,
                                             skip_group_check=True)
                            nc.tensor.matmul(ps_o[h][:, 3 * P:],
                                             vn_sb[kc0 + 1], pr[:, 1, 0:P],
                                             start=False, stop=True,
                                             skip_group_check=True)
                            dtiles[h].append(pr)
                    drain()

                out = []
                # last-window pair 0: drain PV to SBUF immediately so ps_o
                # recycles for pair 1 without waiting the normalize chain
                tn = {}
                if early_free:
                    for h in heads:
                        tn[h] = atp.tile([P, NQ], F32, tag=f"tn{h % 2}",
                                         name=f"tn{h % 2}")
                        nc.vector.tensor_copy(tn[h], ps_o[h])
                for h in heads:
                    d1, d2 = dtiles[h]
                    da = d1[:, 0, :]            # diag accum, in place
                    nc.vector.tensor_add(da[:, P:], da[:, P:],
                                         flat(d1)[:, NQ:NQ + 384])
                    nc.vector.tensor_add(da[:, 2 * P:], da[:, 2 * P:],
                                         d2[:, 0, 0:256])
                    nc.vector.tensor_add(da[:, 3 * P:], da[:, 3 * P:],
                                         d2[:, 1, 0:P])
                    if racc[h] is None:
                        acc = da
                    else:
                        fa = sap.tile([P, NQ], BF, tag="fa", name="fa")
                        nc.vector.tensor_add(fa, racc[h][:, 0, :],
                                             racc[h][:, 1, :])
                        nc.vector.tensor_add(fa, fa, da)
                        acc = fa
                    if mm_den:
                        # denominator as a ones-matmul: every PSUM row gets
                        # sum-over-partitions(acc).  Cheap on PE (512 cy) and
                        # drops the slow Pool all-reduce from the last
                        # window's tail-critical normalize chain.  Borrow a
                        # scores-pool tile (idle once attention is done) so
                        # ps_proj stays free for the final out-proj chunks.
                        den_t = ps_sp.tile([P, 2, NQ], F32, tag="s",
                                           name="s")
                        den = den_t[:, 0, :]
                        nc.tensor.matmul(den, ones_sb, acc, start=True,
                                         stop=True)
                    else:
                        den = bcp.tile([P, NQ], F32, tag="ar", name="ar")
                        nc.gpsimd.partition_all_reduce(den, acc, P,
                                                       bass_isa.ReduceOp.add)
                    bc = bcp.tile([P, NQ], F32, tag="bc", name="bc")
                    nc.vector.reciprocal(bc, den)
                    t = atp.tile([P, NQ], BF, tag=f"t{h % 2}",
                                 name=f"t{h % 2}")
                    nc.vector.tensor_mul(t, tn[h] if early_free else ps_o[h],
                                         bc)
                    ahi = atp.tile([P, NQ], F8, tag=f"ahi{h}", name=f"ahi{h}")
                    nc.scalar.copy(ahi, t)
                    out.append((t, ahi))
                    drain()
                return out

            def attn_half(qs, h0, drain, mm_den=False):
                """256-wide attention half-window for heads h0,h0+1: four
                256-col k-blocks pack into one [P,2,NQ] pair tile (one exp
                per 4 blocks).  Used to split the LAST window so all tail
                ops (normalize chain, out-proj, final DMA) halve, and the
                first half's out-proj overlaps the second half."""
                W2 = 256
                heads = (h0, h0 + 1)
                nf = qs // P                 # full k-blocks before the diag
                ps_o = {h: ps_op.tile([P, NQ], F32, tag="o", name="o")
                        for h in heads}
                packed = {h: [] for h in heads}
                dtile = {}

                def flat(ap):
                    return ap.rearrange("p two c -> p (two c)")

                for s in range(0, nf, 4):
                    e = min(s + 4, nf)
                    for h in heads:
                        pp_s = ps_sp.tile([P, 2, NQ], F32, tag="s", name="s")
                        pr = prb.tile([P, 2, NQ], BF, tag="pr", name="pr")
                        for j, kc in enumerate(range(s, e)):
                            pl, off = j // 2, (j % 2) * W2
                            nc.tensor.matmul(
                                pp_s[:, pl, off:off + W2],
                                kT[:, kc * P:(kc + 1) * P],
                                qkv_sb[h][:, qs:qs + W2],
                                start=True, stop=True)
                        nn = (e - s) * W2
                        nc.scalar.activation(
                            flat(pr)[:, 0:nn], flat(pp_s)[:, 0:nn],
                            mybir.ActivationFunctionType.Exp, scale=EXPSCALE)
                        for j, kc in enumerate(range(s, e)):
                            pl, off = j // 2, (j % 2) * W2
                            nc.tensor.matmul(ps_o[h][:, 0:W2], vn_sb[kc],
                                             pr[:, pl, off:off + W2],
                                             start=(kc == 0), stop=False,
                                             skip_group_check=True)
                        packed[h].append((pr, e - s))
                    drain()
                for h in heads:          # diag pair: widths 256, 128
                    pp_s = ps_sp.tile([P, 2, NQ], F32, tag="s", name="s")
                    pr = prb.tile([P, 2, NQ], BF, tag="pr", name="pr")
                    nc.tensor.matmul(pp_s[:, 0, 0:W2],
                                     kT[:, nf * P:(nf + 1) * P],
                                     qkv_sb[h][:, qs:qs + W2],
                                     start=True, stop=True)
                    nc.tensor.matmul(pp_s[:, 1, 0:P],
                                     kT[:, (nf + 1) * P:(nf + 2) * P],
                                     qkv_sb[h][:, qs + P:qs + W2],
                                     start=True, stop=True)
                    nc.scalar.activation(pr[:, 0, 0:W2], pp_s[:, 0, 0:W2],
                                         mybir.ActivationFunctionType.Exp,
                                         scale=EXPSCALE)
                    nc.scalar.activation(pr[:, 1, 0:P], pp_s[:, 1, 0:P],
                                         mybir.ActivationFunctionType.Exp,
                                         scale=EXPSCALE)
                    for i in range(2):
                        nc.vector.tensor_mul(pr[:, i, 0:P], pr[:, i, 0:P],
                                             tri_sb)
                    nc.tensor.matmul(ps_o[h][:, 0:W2], vn_sb[nf],
                                     pr[:, 0, 0:W2], start=False, stop=False,
                                     skip_group_check=True)
                    nc.tensor.matmul(ps_o[h][:, P:W2], vn_sb[nf + 1],
                                     pr[:, 1, 0:P], start=False, stop=True,
                                     skip_group_check=True)
                    dtile[h] = pr
                    drain()
                out = []
                for h in heads:
                    ft = [flat(pr) for (pr, nb) in packed[h] if nb == 4]
                    pt = [flat(pr)[:, 0:2 * W2]
                          for (pr, nb) in packed[h] if nb == 2]
                    sa = sap.tile([P, 2, NQ], BF, tag="sa", name="sa")
                    fs = flat(sa)
                    nc.vector.tensor_add(fs, ft[0], ft[1])
                    for x in ft[2:]:
                        nc.vector.tensor_add(fs, fs, x)
                    for x in pt:
                        nc.vector.tensor_add(fs[:, 0:2 * W2],
                                             fs[:, 0:2 * W2], x)
                    d = dtile[h]
                    da = d[:, 0, :]
                    nc.vector.tensor_add(da[:, P:W2], da[:, P:W2],
                                         d[:, 1, 0:P])
                    fa = sap.tile([P, NQ], BF, tag="fa", name="fa")
                    fw = fa[:, 0:W2]
                    nc.vector.tensor_add(fw, sa[:, 0, 0:W2],
                                         sa[:, 0, W2:2 * W2])
                    nc.vector.tensor_add(fw, fw, sa[:, 1, 0:W2])
                    nc.vector.tensor_add(fw, fw, sa[:, 1, W2:2 * W2])
                    nc.vector.tensor_add(fw, fw, da[:, 0:W2])
                    if mm_den:
                        den_t = ps_sp.tile([P, 2, NQ], F32, tag="s",
                                           name="s")
                        den = den_t[:, 0, 0:W2]
                        nc.tensor.matmul(den, ones_sb, fw, start=True,
                                         stop=True)
                    else:
                        den_f = bcp.tile([P, NQ], F32, tag="ar", name="ar")
                        den = den_f[:, 0:W2]
                        nc.gpsimd.partition_all_reduce(den, fw, P,
                                                       bass_isa.ReduceOp.add)
                    bc = bcp.tile([P, NQ], F32, tag="bc", name="bc")
                    nc.vector.reciprocal(bc[:, 0:W2], den)
                    t = atp.tile([P, NQ], BF, tag=f"t{h % 2}",
                                 name=f"t{h % 2}")
                    nc.vector.tensor_mul(t[:, 0:W2], ps_o[h][:, 0:W2],
                                         bc[:, 0:W2])
                    ahi = atp.tile([P, NQ], F8, tag=f"ahi{h}", name=f"ahi{h}")
                    nc.scalar.copy(ahi[:, 0:W2], t[:, 0:W2])
                    out.append((t, ahi))
                    drain()
                return out

            # ---------------- main window loop ----------------
            from collections import deque

            def make_drain(fillers, nsteps, boost=1.0):
                st = {"a": 0.0, "r": len(fillers) / nsteps * boost}

                def drain():
                    st["a"] += st["r"]
                    while fillers and st["a"] >= 1.0:
                        st["a"] -= 1.0
                        fillers.popleft()()
                return drain

            for m in (0, 1, 4, 5, 2, 3):
                proj_window(0, m)
            vnat_window(0)
            prev = None       # (ahi list, alo list) of previous window
            for w in range(NW):
                if w + 1 < NW:
                    load_x_window(w + 1)
                if w == 0:
                    load_wo()
                fillers = deque()
                if prev is not None:
                    pv = prev
                    for me in range(16):
                        fillers.append(
                            lambda me=me, pv=pv: outproj_chunk(
                                w - 1, me, pv[0], pv[1]))
                if w + 1 < NW:
                    for m in (0, 1, 4, 5, 2, 3):
                        fillers.append(
                            lambda m=m, w1=w + 1: proj_window(w1, m))
                    fillers.append(lambda w1=w + 1: vnat_window(w1))
                nsteps = 2 * (2 * w + 2) + 4
                last = w == NW - 1
                drain = make_drain(fillers, nsteps, boost=1.0)

                def finish_heads(res, W):
                    ahi_l, alo_l = [], [None, None]
                    for h, (t, ahi) in enumerate(res):
                        if h % 2 == 0:
                            alo_l[h // 2] = atp.tile(
                                [P, 2, NQ], F8, tag=f"alo{h // 2}",
                                name=f"alo{h // 2}")
                        nc.vector.scalar_tensor_tensor(
                            alo_l[h // 2][:, h % 2, 0:W], t[:, 0:W], 1.0,
                            ahi[:, 0:W],
                            mybir.AluOpType.mult, mybir.AluOpType.subtract)
                        ahi_l.append(ahi)
                    return ahi_l, alo_l

                if not last:
                    res = []
                    for h0 in (0, 2):
                        res += attn_pair(w, h0, drain)
                    prev = finish_heads(res, NQ)
                else:
                    # last window: two 256-wide halves.  The A half's
                    # out-proj becomes PE filler work for the B half, and
                    # the tail-critical normalize/out-proj/DMA all halve.
                    resA = []
                    for h0 in (0, 2):
                        resA += attn_half(S - 2 * 256, h0, drain)
                    ahiA, aloA = finish_heads(resA, 256)
                    for me0 in range(0, 16, 2):
                        fillers.append(
                            lambda me0=me0: outproj_pair256(
                                S - 2 * 256, me0, ahiA, aloA))
                    resB = []
                    for h0 in (0, 2):
                        resB += attn_half(S - 256, h0, drain, mm_den=True)
                    prev = finish_heads(resB, 256)
                while fillers:
                    fillers.popleft()()
            for me0 in range(0, 14, 2):
                outproj_pair256(S - 256, me0, prev[0], prev[1])
            for me in (14, 15):
                outproj_chunkW(S - 256, 256, me, prev[0], prev[1],
                               single=True)

    nc.finalize()
    # The standard compile pipeline leaves the (unused) register preamble
    # in place here, which the walrus birverifier then rejects with "Reg
    # has not been allocated yet"; a second DCE pass removes it.
    nc.dce_regs()
    return nc


def _get_nc():
    if "nc" not in _CACHE:
        _CACHE["nc"] = _build()
    return _CACHE["nc"]


def _hi_lo(a):
    hi = a.astype(E4)
    lo = (a - hi.astype(np.float32)).astype(E4)
    return hi, lo


def kernel(x, Wq, Wk, Wv, Wo, _trace=False, _tmpdir=None):
    x = np.asarray(x, np.float32)
    Wq, Wk, Wv, Wo = (np.asarray(a, np.float32) for a in (Wq, Wk, Wv, Wo))
    nc = _get_nc()
    identq = np.eye(P, dtype=np.float32).astype(BF_NP)
    tri = np.triu(np.ones((P, P), np.float32)).astype(BF_NP)
    from concurrent.futures import ThreadPoolExecutor

    def _xprep(b):
        xT = np.ascontiguousarray(x[b].T)
        return _hi_lo(xT)

    with ThreadPoolExecutor(8) as tp:
        xp = list(tp.map(_xprep, range(B)))

        def _core(c):
            b, g = c // 4, c % 4
            wqkv = np.concatenate(
                [Wq[512 * g:512 * (g + 1)],
                 Wk[128 * g:128 * (g + 1)],
                 Wv[128 * g:128 * (g + 1)]], axis=0) * WS
            wqkvT = np.ascontiguousarray(wqkv.T)
            whi, wlo = _hi_lo(wqkvT)
            wmain = np.ascontiguousarray(np.stack([whi, wlo], axis=1))
            woT = np.ascontiguousarray(Wo[:, 512 * g:512 * (g + 1)].T) * WS
            wohi, wolo = _hi_lo(woT)
            womain = np.ascontiguousarray(np.stack([wohi, wolo], axis=1))
            return {
                "xhiT": xp[b][0], "xloT": xp[b][1],
                "wmain": wmain, "womain": womain,
                "identq": identq, "tri": tri,
            }
        in_maps = list(tp.map(_core, range(8)))
    res = bass_utils.run_bass_kernel_spmd(
        nc, in_maps, core_ids=list(range(8)), trace=_trace, tmpdir=_tmpdir)
    out = np.zeros((B, S, E), np.float32)
    for c in range(8):
        out[c // 4] += res.results[c]["outT"].astype(np.float32).T
    out *= OUTSCALE
    if _trace:
        return out, res
    return out

